# revision 5
# baseline (speedup 1.0000x reference)
"""Trainium2 Bass kernel for BatchRemoveQuatDiscontinuities.

Algorithm (per (batch, joint) lane):
    d[t]    = dot(q[t], q[t-1])                (fp32, 4-wide dot)
    flip[t] = 1 if d[t] < 0 else 0             (t >= 1; flip[0] = 0)
    sigma[t] = (-1)^(sum_{s<=t} flip[s])       (cumulative sign parity)
    out[t]  = q[t] * sigma[t]

Mapping on a NeuronCore (data-parallel over batch across 8 cores):
  * One tile = CP=2 batch clips, loaded as a single fully-contiguous 2MB
    DMA: [128 partitions = t/8, free = (clip: 2, ts: 8, j: 64, c: 4)].
    Loads issue on the SP HWDGE ring (nc.sync), stores on the ACT ring
    (nc.scalar) so the two streams drain concurrently and per-DMA
    completion latency of one ring hides under the other.
  * q[t-1]: within a partition it is a free-axis offset (-256); the
    octet boundary (ts=0) needs q[p-1, ts=7], produced by a TensorE
    matmul with an off-diagonal 0/1 matrix S into PSUM (fp32 exact).
    Row 0 of S is zero => d[t=0] = +0.0 => no flip at t=0 for free.
  * prod on DVE; pairwise 4-dot: u=(c0+c1, c2+c3) on GpSimd, d=u0+u1 on
    DVE written in (j, ts) order; flip e = (d < 0) via one tensor_scalar.
  * Within-octet inclusive prefix parity: tensor_tensor_scan with a
    reset mask (state = mask*state xor e), segments of 8.  Octet-level
    count of odd rows above via strict-triangular matmul on the ts=7
    slice; parity of the count -> sigo in {+-1} (int &1, affine).
  * sigma_row = 1-2*rowp (one 2-op tensor_scalar, bf16); sig = sigr*sigo
    on GpSimd.  Final out = q * sig (broadcast over c) split DVE/GpSimd
    by the mult_split knob - exact +/-1 multiply.
"""

import numpy as np
from contextlib import ExitStack

import concourse.bass as bass
import concourse.bacc as bacc
import concourse.tile as tile
from concourse import mybir
from concourse.bass_utils import run_bass_kernel_spmd

B, T, J, C = 128, 1024, 64, 4
NCORES = 8
JC = J * C                      # 256 floats per t
BPC = B // NCORES               # 16 batch clips per core
TS = 8                          # t per partition (octet)
FD = TS * JC                    # per-clip free dim = 2048 floats
SD = J * TS                     # per-clip prefix free dim = 512 (j, ts)
CP = 2                          # clips per tile (2MB DMAs)
NT = BPC // CP                  # tiles per core

FP32 = mybir.dt.float32
BF16 = mybir.dt.bfloat16
I32 = mybir.dt.int32
Alu = mybir.AluOpType


def _ap(apx, dims):
    """AP with explicit [step, count] free dims appended to partition dim."""
    return bass.AP(
        tensor=apx.tensor, offset=apx.offset,
        ap=[list(apx.ap[0]), *[list(d) for d in dims]],
    )


def build_nc(bpc=BPC, t=T, reps=1, mode="full", mult_split=2):
    assert t % (128 * TS) == 0
    nc = bacc.Bacc(None, target_bir_lowering=False)
    q = nc.declare_dram_parameter("q", [bpc, t, J, C], FP32, isOutput=False)
    smat = nc.declare_dram_parameter("smat", [128, 128], FP32, isOutput=False)
    pmat = nc.declare_dram_parameter("pmat", [128, 128], FP32, isOutput=False)
    out = nc.declare_dram_parameter("out", [bpc, t, J, C], FP32, isOutput=True)
    qf = q.rearrange("b t j c -> b (t j c)")
    of = out.rearrange("b t j c -> b (t j c)")

    with tile.TileContext(nc) as tc, ExitStack() as ctx:
        consts = ctx.enter_context(tc.tile_pool(name="consts", bufs=1))
        qpool = ctx.enter_context(tc.tile_pool(name="qpool", bufs=4))
        opool = ctx.enter_context(tc.tile_pool(name="opool", bufs=3))
        spool = ctx.enter_context(tc.tile_pool(name="spool", bufs=3))
        auxp = ctx.enter_context(tc.tile_pool(name="auxp", bufs=3, space="PSUM"))
        offp = ctx.enter_context(tc.tile_pool(name="offp", bufs=3, space="PSUM"))

        smatSB = consts.tile([128, 128], FP32)
        nc.sync.dma_start(out=smatSB[:, :], in_=smat[:, :])
        pmatSB = consts.tile([128, 128], FP32)
        nc.sync.dma_start(out=pmatSB[:, :], in_=pmat[:, :])
        amask = consts.tile([128, CP * SD], FP32)
        nc.vector.memset(amask[:, :], 1.0)
        nc.vector.memset(
            amask.rearrange("p (b j ts) -> p b j ts", b=CP, ts=TS)[:, :, :, 0],
            0.0,
        )

        def emit_body():
            for g in range(NT):
                emit_tile(g)

        def emit_tile(g):
            qt = qpool.tile([128, CP, FD], FP32, tag="qt")
            nc.sync.dma_start(
                out=qt[:, :, :],
                in_=qf[g * CP:(g + 1) * CP, :].rearrange(
                    "b (p x) -> p b x", p=128
                ),
            )
            o = opool.tile([128, CP, FD], FP32, tag="o")
            if mode == "dma":
                nc.scalar.dma_start(
                    out=of[g * CP:(g + 1) * CP, :].rearrange(
                        "b (p x) -> p b x", p=128
                    ),
                    in_=qt[:, :, :],
                )
                return

            # octet-boundary shift: aux[p, b] = qt[p-1, b, ts=7 chunk]
            aux = auxp.tile([128, CP, JC], FP32, tag="aux")
            nc.tensor.matmul(
                aux[:, :, :],
                lhsT=smatSB[:, :],
                rhs=qt[:, :, FD - JC:FD],
                start=True,
                stop=True,
            )

            # prod: o = q * q_shifted  (DVE)
            nc.vector.tensor_tensor(
                out=o[:, :, JC:FD], in0=qt[:, :, JC:FD], in1=qt[:, :, 0:FD - JC],
                op=Alu.mult,
            )
            nc.vector.tensor_tensor(
                out=o[:, :, 0:JC], in0=qt[:, :, 0:JC], in1=aux[:, :, :],
                op=Alu.mult,
            )

            # dot over c: u = (c0+c1, c2+c3) on GpSimd, d = u0+u1 on DVE,
            # d written in (j, ts) order per clip
            u = spool.tile([128, CP, 2 * SD], FP32, tag="u")
            d = spool.tile([128, CP, SD], FP32, tag="d")
            for c2 in range(CP):
                opairs = o.rearrange(
                    "p b (s k two) -> p b s k two", k=2, two=2
                )[:, c2]
                uv = u.rearrange("p b (s k) -> p b s k", k=2)[:, c2]
                nc.gpsimd.tensor_tensor(
                    out=uv, in0=opairs[:, :, :, 0], in1=opairs[:, :, :, 1],
                    op=Alu.add,
                )
                u_k = u.rearrange(
                    "p b (ts j k) -> p b ts j k", j=J, k=2
                )[:, c2]
                dv = bass.AP(
                    tensor=d.tensor, offset=d.offset + c2 * SD,
                    ap=[list(d.ap[0]), [1, TS], [TS, J]],
                )
                nc.vector.tensor_tensor(
                    out=dv, in0=u_k[:, :, :, 0], in1=u_k[:, :, :, 1],
                    op=Alu.add,
                )

            # flip indicator e = (d < 0) in {0.0, 1.0}; d[t=0] = +0.0 -> 0
            df = d.rearrange("p b s -> p (b s)")
            e = spool.tile([128, CP * SD], FP32, tag="e")
            nc.vector.tensor_scalar(
                out=e[:, :], in0=df, scalar1=0.0, scalar2=None, op0=Alu.is_lt,
            )

            # within-octet inclusive prefix parity (segmented xor-scan)
            rowp = spool.tile([128, CP * SD], FP32, tag="rowp")
            nc.vector.tensor_tensor_scan(
                out=rowp[:, :], data0=amask[:, :], data1=e[:, :],
                initial=0.0, op0=Alu.mult, op1=Alu.logical_xor,
            )

            # octet-level: count of odd rows above (parity-sum via matmul)
            offs = offp.tile([128, CP, J], FP32, tag="offs")
            rr = rowp.rearrange("p (b j ts) -> p b j ts", b=CP, ts=TS)
            nc.tensor.matmul(
                offs[:, :, :],
                lhsT=pmatSB[:, :],
                rhs=rr[:, :, :, 7],
                start=True,
                stop=True,
            )
            # parity of count -> sigo in {+1, -1} (bf16)
            offi = spool.tile([128, CP * J], I32, tag="offi")
            nc.vector.tensor_copy(
                out=offi[:, :], in_=offs.rearrange("p b j -> p (b j)"),
            )
            offb = spool.tile([128, CP * J], I32, tag="offb")
            nc.vector.tensor_scalar(
                out=offb[:, :], in0=offi[:, :], scalar1=1, scalar2=None,
                op0=Alu.bitwise_and,
            )
            sigo = spool.tile([128, CP * J], BF16, tag="sigo")
            nc.vector.tensor_scalar(
                out=sigo[:, :], in0=offb[:, :], scalar1=-2.0, scalar2=1.0,
                op0=Alu.mult, op1=Alu.add,
            )
            # sigma_row in {+1, -1} from the 0/1 row parity (one 2-op TS)
            sigr = spool.tile([128, CP * SD], BF16, tag="sigr")
            nc.vector.tensor_scalar(
                out=sigr[:, :], in0=rowp[:, :], scalar1=-2.0, scalar2=1.0,
                op0=Alu.mult, op1=Alu.add,
            )
            # sigma = sigma_row * sigma_off, (b, j, ts) layout  (GpSimd)
            sig = spool.tile([128, CP * SD], BF16, tag="sig")
            nc.gpsimd.tensor_tensor(
                out=sig.rearrange("p (bj ts) -> p bj ts", ts=TS),
                in0=sigr.rearrange("p (bj ts) -> p bj ts", ts=TS),
                in1=_ap(sigo, [[1, CP * J], [0, TS]]),
                op=Alu.mult,
            )

            # out = q * sigma (broadcast over c), exact +/-1 multiply;
            # ts-range [0, mult_split) on DVE, rest on GpSimd
            for c2 in range(CP):
                qv = qt.rearrange("p b (ts x) -> p b ts x", ts=TS)[:, c2]
                ow = o.rearrange("p b (ts x) -> p b ts x", ts=TS)[:, c2]
                sbase = sig.offset + c2 * SD
                if mult_split > 0:
                    nc.vector.tensor_tensor(
                        out=ow[:, 0:mult_split, :],
                        in0=qv[:, 0:mult_split, :],
                        in1=bass.AP(
                            tensor=sig.tensor, offset=sbase,
                            ap=[list(sig.ap[0]), [1, mult_split], [TS, J],
                                [0, C]],
                        ),
                        op=Alu.mult,
                    )
                if mult_split < TS:
                    nc.gpsimd.tensor_tensor(
                        out=ow[:, mult_split:TS, :],
                        in0=qv[:, mult_split:TS, :],
                        in1=bass.AP(
                            tensor=sig.tensor, offset=sbase + mult_split,
                            ap=[list(sig.ap[0]), [1, TS - mult_split],
                                [TS, J], [0, C]],
                        ),
                        op=Alu.mult,
                    )

            nc.scalar.dma_start(
                out=of[g * CP:(g + 1) * CP, :].rearrange(
                    "b (p x) -> p b x", p=128
                ),
                in_=o[:, :, :],
            )

        if reps == 1:
            emit_body()
        else:
            with tc.For_i(0, reps, 1):
                emit_body()
    return nc


def make_consts():
    smat = np.eye(128, k=1, dtype=np.float32)       # S[k, m] = 1 iff m == k+1
    pmat = np.triu(np.ones((128, 128), np.float32), k=1)  # strict prefix
    return smat, pmat


def make_in_maps(q, smat, pmat):
    return [
        {"q": q[c * BPC:(c + 1) * BPC], "smat": smat, "pmat": pmat}
        for c in range(NCORES)
    ]


def kernel(joint_rotations: np.ndarray) -> np.ndarray:
    q = np.ascontiguousarray(joint_rotations, dtype=np.float32)
    assert q.shape == (B, T, J, C)
    smat, pmat = make_consts()
    nc = build_nc()
    nc.finalize()   # run bacc passes (wait splitting, reg alloc) + freeze
    in_maps = make_in_maps(q, smat, pmat)
    res = run_bass_kernel_spmd(nc, in_maps, list(range(NCORES)))
    outs = [np.asarray(r["out"]) for r in res.results]
    return np.concatenate(outs, axis=0)


# revision 16
# speedup vs baseline: 1.4487x; 1.4487x over previous
"""Trainium2 Bass kernel for BatchRemoveQuatDiscontinuities.

Algorithm (per (batch, joint) lane):
    d[t]    = dot(q[t], q[t-1])                (fp32, 4-wide dot)
    flip[t] = 1 if d[t] < 0 else 0             (t >= 1; flip[0] = 0)
    sigma[t] = (-1)^(sum_{s<=t} flip[s])       (cumulative sign parity)
    out[t]  = q[t] * sigma[t]

Mapping on a NeuronCore (data-parallel over batch across 8 cores):
  * One tile = one batch clip, loaded as a single fully-contiguous 1MB
    DMA: [128 partitions = t/8, free = (ts: 8, j: 64, c: 4)].  Loads on
    the SP HWDGE ring (nc.sync), stores on the ACT ring (nc.scalar).
  * q[t-1]: within a partition it is a free-axis offset (-256); the
    octet boundary (ts=0) needs q[p-1, ts=7], produced by a TensorE
    matmul with an off-diagonal 0/1 matrix S into PSUM (fp32 exact).
  * prod on DVE, 4-wide dot via two pairwise adds (c0+c1)+(c2+c3),
    d written in (j, ts) order; flip indicator e = Relu(Sign(-d)) on
    ScalarE (bf16).
  * Within-octet inclusive prefix: tensor_tensor_scan with a reset mask
    (state = mask*state xor e), segments of 8 per joint.  Octet-level
    parity via strict-triangular matmul over partitions on the per-row
    totals; parity of the count -> sigo (int &1, ACT affine to +-1).
  * sigma_row = 1-2*rowp on ScalarE (bf16); sig = sigr*sigo on GpSimd.
    Final out = q * sig (broadcast over c), ts-split DVE/GpSimd by
    mult_split - exact +/-1 multiply.

Engine budget (HW-measured): DVE runs ~1.4ns/elem fp32 TT and is the
critical engine; GpSimd is ~4-6x slower than its cost model and only
tolerates ~16-24K elems/rep; ScalarE absorbs the activation chain; the
DMA floor (in+out on the two HWDGE rings) is ~103.5us/rep.
"""

import numpy as np
from contextlib import ExitStack

import concourse.bass as bass
import concourse.bacc as bacc
import concourse.tile as tile
from concourse import mybir
from concourse.bass_utils import run_bass_kernel_spmd

B, T, J, C = 128, 1024, 64, 4
NCORES = 8
JC = J * C                      # 256 floats per t
BPC = B // NCORES               # 16 batch clips per core
TS = 8                          # t per partition (octet)
FD = TS * JC                    # tile free dim = 2048 floats
SD = J * TS                     # prefix free dim = 512 (j, ts)

FP32 = mybir.dt.float32
BF16 = mybir.dt.bfloat16
I32 = mybir.dt.int32
Alu = mybir.AluOpType
Act = mybir.ActivationFunctionType


def _ap(apx, dims):
    """AP with explicit [step, count] free dims appended to partition dim."""
    return bass.AP(
        tensor=apx.tensor, offset=apx.offset,
        ap=[list(apx.ap[0]), *[list(d) for d in dims]],
    )


def build_nc(bpc=BPC, t=T, reps=1, mode="full", mult_split=4,
             sig_eng="dve", d_eng="dve", u_eng="dve", scan_eng="dve",
             out_ring="sync", qbufs=8, obufs=5, sbufs=4, sigo_cos=0):
    assert t % (128 * TS) == 0
    nc = bacc.Bacc(None, target_bir_lowering=False)
    q = nc.declare_dram_parameter("q", [bpc, t, J, C], FP32, isOutput=False)
    smat = nc.declare_dram_parameter("smat", [128, 128], FP32, isOutput=False)
    pmat = nc.declare_dram_parameter("pmat", [128, 128], FP32, isOutput=False)
    out = nc.declare_dram_parameter("out", [bpc, t, J, C], FP32, isOutput=True)
    qf = q.rearrange("b t j c -> b (t j c)")
    of = out.rearrange("b t j c -> b (t j c)")

    eng_sig = nc.gpsimd if sig_eng == "pool" else nc.vector
    eng_d = nc.gpsimd if d_eng == "pool" else nc.vector
    eng_u = nc.gpsimd if u_eng == "pool" else nc.vector
    eng_scan = nc.gpsimd if scan_eng == "pool" else nc.vector
    eng_out = nc.scalar if out_ring == "act" else nc.sync

    with tile.TileContext(nc) as tc, ExitStack() as ctx:
        consts = ctx.enter_context(tc.tile_pool(name="consts", bufs=1))
        qpool = ctx.enter_context(tc.tile_pool(name="qpool", bufs=qbufs))
        opool = ctx.enter_context(tc.tile_pool(name="opool", bufs=obufs))
        spool = ctx.enter_context(tc.tile_pool(name="spool", bufs=sbufs))
        auxp = ctx.enter_context(tc.tile_pool(name="auxp", bufs=4, space="PSUM"))
        offp = ctx.enter_context(tc.tile_pool(name="offp", bufs=4, space="PSUM"))

        smatSB = consts.tile([128, 128], FP32)
        nc.sync.dma_start(out=smatSB[:, :], in_=smat[:, :])
        pmatSB = consts.tile([128, 128], FP32)
        nc.sync.dma_start(out=pmatSB[:, :], in_=pmat[:, :])
        amask = consts.tile([128, SD], FP32)
        nc.vector.memset(amask[:, :], 1.0)
        nc.vector.memset(
            amask.rearrange("p (j ts) -> p j ts", ts=TS)[:, :, 0], 0.0
        )
        pihalf = consts.tile([128, 1], FP32)
        nc.vector.memset(pihalf[:, :], float(np.pi / 2))

        def emit_body():
            for b in range(bpc):
                emit_tile(b)

        def emit_tile(b):
            qt = qpool.tile([128, FD], FP32, tag="qt")
            nc.sync.dma_start(
                out=qt[:, :],
                in_=qf[b, :].rearrange("(p x) -> p x", p=128),
            )
            o = opool.tile([128, FD], FP32, tag="o")
            if mode == "dma":
                eng_out.dma_start(
                    out=of[b, :].rearrange("(p x) -> p x", p=128), in_=qt[:, :]
                )
                return

            # octet-boundary shift: aux[p] = qt[p-1, ts=7 chunk] (row 0 = 0)
            aux = auxp.tile([128, JC], FP32, tag="aux")
            nc.tensor.matmul(
                aux[:, :],
                lhsT=smatSB[:, :],
                rhs=qt[:, FD - JC:FD],
                start=True,
                stop=True,
            )

            # prod: o = q * q_shifted  (DVE)
            nc.vector.tensor_tensor(
                out=o[:, JC:FD], in0=qt[:, JC:FD], in1=qt[:, 0:FD - JC],
                op=Alu.mult,
            )
            nc.vector.tensor_tensor(
                out=o[:, 0:JC], in0=qt[:, 0:JC], in1=aux[:, :], op=Alu.mult,
            )

            # dot over c, pairwise (c0+c1)+(c2+c3); d written in (j, ts) order
            u = spool.tile([128, 2 * SD], FP32, tag="u")
            ov = o.rearrange("p (s c) -> p s c", c=C)
            uv = u.rearrange("p (s k) -> p s k", k=2)
            opairs = ov.rearrange("p s (k two) -> p s k two", k=2)
            eng_u.tensor_tensor(
                out=uv, in0=opairs[:, :, :, 0], in1=opairs[:, :, :, 1],
                op=Alu.add,
            )
            d = spool.tile([128, SD], FP32, tag="d")  # (j, ts) layout
            u_k = u.rearrange("p (ts j k) -> p ts j k", j=J, k=2)
            eng_d.tensor_tensor(
                out=_ap(d, [[1, TS], [TS, J]]),
                in0=u_k[:, :, :, 0],
                in1=u_k[:, :, :, 1],
                op=Alu.add,
            )

            # flip indicator e = Relu(Sign(-d)), bf16, (j, ts) layout  (ACT)
            sg = spool.tile([128, SD], FP32, tag="sg")
            nc.scalar.activation(sg[:, :], d[:, :], Act.Sign, scale=-1.0)
            e = spool.tile([128, SD], BF16, tag="e")
            nc.scalar.activation(e[:, :], sg[:, :], Act.Relu)
            # t=0 has no flip (also guards Sign(0) semantics)
            nc.scalar.mul(
                e.rearrange("p (j ts) -> p j ts", ts=TS)[0:1, :, 0],
                e.rearrange("p (j ts) -> p j ts", ts=TS)[0:1, :, 0],
                0.0,
            )

            # within-octet inclusive prefix PARITY (segmented xor-scan):
            # state = (mask * state) xor e  -> 0/1 running parity per joint
            rowp = spool.tile([128, SD], FP32, tag="rowp")
            eng_scan.tensor_tensor_scan(
                out=rowp[:, :], data0=amask[:, :], data1=e[:, :],
                initial=0.0, op0=Alu.mult, op1=Alu.logical_xor,
            )

            # octet-level: count of odd rows above (parity-sum via matmul)
            offs = offp.tile([128, J], FP32, tag="offs")
            nc.tensor.matmul(
                offs[:, :],
                lhsT=pmatSB[:, :],
                rhs=rowp.rearrange("p (j ts) -> p j ts", ts=TS)[:, :, 7],
                start=True,
                stop=True,
            )
            # parity of that count -> sigma_off in {+1, -1} per (p, j)
            sigo = spool.tile([128, J], BF16, tag="sigo")
            if sigo_cos:
                # count mod 2 on DVE (exact: counts are integer-valued fp32),
                # then the +-1 affine on ACT.
                offm = spool.tile([128, J], FP32, tag="offm")
                nc.vector.tensor_scalar(
                    out=offm[:, :], in0=offs[:, :], scalar1=2.0, scalar2=None,
                    op0=Alu.mod,
                )
                nc.scalar.activation(sigo[:, :], offm[:, :], Act.Copy,
                                     bias=1.0, scale=-2.0)
            else:
                offi = spool.tile([128, J], I32, tag="offi")
                nc.vector.tensor_copy(out=offi[:, :], in_=offs[:, :])
                offb = spool.tile([128, J], I32, tag="offb")
                nc.vector.tensor_scalar(
                    out=offb[:, :], in0=offi[:, :], scalar1=1, scalar2=None,
                    op0=Alu.bitwise_and,
                )
                nc.scalar.activation(sigo[:, :], offb[:, :], Act.Copy,
                                     bias=1.0, scale=-2.0)
            # sigma_row in {+1, -1} from the 0/1 row parity  (ACT)
            sigr = spool.tile([128, SD], BF16, tag="sigr")
            nc.scalar.activation(sigr[:, :], rowp[:, :], Act.Copy,
                                 bias=1.0, scale=-2.0)
            # sigma = sigma_row * sigma_off, (j, ts) layout
            sig = spool.tile([128, SD], BF16, tag="sig")
            eng_sig.tensor_tensor(
                out=sig.rearrange("p (j ts) -> p j ts", ts=TS),
                in0=sigr.rearrange("p (j ts) -> p j ts", ts=TS),
                in1=_ap(sigo, [[1, J], [0, TS]]),
                op=Alu.mult,
            )

            # out = q * sigma (broadcast over c), exact +/-1 multiply;
            # split by ts-range between VectorE and GpSimd
            qv = qt.rearrange("p (ts x) -> p ts x", ts=TS)
            ow = o.rearrange("p (ts x) -> p ts x", ts=TS)
            tsplit = mult_split
            if tsplit > 0:
                nc.vector.tensor_tensor(
                    out=ow[:, 0:tsplit, :],
                    in0=qv[:, 0:tsplit, :],
                    in1=bass.AP(
                        tensor=sig.tensor, offset=sig.offset,
                        ap=[list(sig.ap[0]), [1, tsplit], [TS, J], [0, C]],
                    ),
                    op=Alu.mult,
                )
            if tsplit < TS:
                nc.gpsimd.tensor_tensor(
                    out=ow[:, tsplit:TS, :],
                    in0=qv[:, tsplit:TS, :],
                    in1=bass.AP(
                        tensor=sig.tensor, offset=sig.offset + tsplit,
                        ap=[list(sig.ap[0]), [1, TS - tsplit], [TS, J],
                            [0, C]],
                    ),
                    op=Alu.mult,
                )

            eng_out.dma_start(
                out=of[b, :].rearrange("(p x) -> p x", p=128), in_=o[:, :]
            )

        if reps == 1:
            emit_body()
        else:
            with tc.For_i(0, reps, 1):
                emit_body()
    return nc


def make_consts():
    smat = np.eye(128, k=1, dtype=np.float32)       # S[k, m] = 1 iff m == k+1
    pmat = np.triu(np.ones((128, 128), np.float32), k=1)  # strict prefix
    return smat, pmat


def make_in_maps(q, smat, pmat):
    return [
        {"q": q[c * BPC:(c + 1) * BPC], "smat": smat, "pmat": pmat}
        for c in range(NCORES)
    ]


def kernel(joint_rotations: np.ndarray) -> np.ndarray:
    q = np.ascontiguousarray(joint_rotations, dtype=np.float32)
    assert q.shape == (B, T, J, C)
    smat, pmat = make_consts()
    nc = build_nc()
    nc.finalize()   # run bacc passes (wait splitting, reg alloc) + freeze
    in_maps = make_in_maps(q, smat, pmat)
    res = run_bass_kernel_spmd(nc, in_maps, list(range(NCORES)))
    outs = [np.asarray(r["out"]) for r in res.results]
    return np.concatenate(outs, axis=0)
